# revision 2
# baseline (speedup 1.0000x reference)
"""Cubic B-spline interpolation kernel for Trainium2 (Bass/Tile), 8 cores.

Reference: for each of 2M points, evaluate a cardinal cubic B-spline on a
132^3 control grid (4x4x4 stencil per point).  setup_inputs draws
pts = uniform[0,1)*128, so t = pts+1 in [1,129) and all stencil bases
bx,by,bz lie in 1..128 -> zero-based bases x0,y0,z0 in 0..127.

Strategy (data-parallel over points, grid replicated per core):
  - Host: shard points into 8 slices of 250,000, pad to 250,880 =
    128 partitions x 1960 slots; ship the raw grid transposed to
    gt[y][x][z] (9.2 MB).
  - Device phase 1 (table build): construct a 16x-interleaved table
    T16[x0][y0][z][i][j] = G[x0+i][y0+j][z], dims [128][128][131][4][4]
    f32 (137 MB DRAM scratch).  32 blocks of 4 x-planes: 4 partition-
    shifted DMA loads (DMA crosses partitions; DVE cannot), 16
    lane-aligned DVE interleave copies into staging S[y0][b][z][ij],
    4 DMA stores.  With this layout one contiguous 64-float (256B) run
    at ((bx-1)*128 + (by-1))*2096 + (bz-1)*16 covers a point's whole
    4x4x4 stencil: the z-taps come from the sliding window over z.
  - Device phase 2 (gather+contract), 49 chunks x 40 slots: floor/frac/
    weights on DVE, flat index, one 256B indirect-DMA descriptor per
    point (the n_idx=1 form; batched offsets are broken in this stack,
    and runs >4KB hit a pathological slow path - 6400B descriptors ran
    8x slower).  Tensor-product contraction on DVE: reduce j (y), then
    i (x), then k (z).
  - Output [128 x 1960] per core; host unshards/unpads.
"""

from contextlib import ExitStack

import numpy as np

GRID = 132
G2 = GRID * GRID  # 17424
P = 128
SLOTS = 1960
NC = 40
NCHUNK = SLOTS // NC  # 49
NPTS_CORE = 250_000
NPAD_CORE = P * SLOTS  # 250880

NX = 128            # x0/y0 base range (bases 0..127)
NZ = 131            # z rows in table (z0 max 127 reads z0..z0+3 -> 130)
CELL = 16           # (i,j) interleave factor
ROW = NZ * CELL     # 2096 elements per (x0,y0) cell row
PLANE = NX * ROW    # 268288 elements per x0
TSIZE = NX * PLANE  # 34,340,864 elements = 137.4 MB

_CACHE = {}


def _build_program(nchunks=NCHUNK):
    """nchunks > NCHUNK repeats gather chunks modulo NCHUNK (timing use)."""
    import concourse.bass as bass
    import concourse.tile as tile
    from concourse import bacc, mybir

    nc = bacc.Bacc("TRN2", num_devices=8, debug=False, target_bir_lowering=False)
    pts_d = nc.dram_tensor("pts", [NPAD_CORE, 3], mybir.dt.float32, kind="ExternalInput")
    gt_d = nc.dram_tensor("gt", [GRID * G2, 1], mybir.dt.float32, kind="ExternalInput")
    out_d = nc.dram_tensor("out", [P, SLOTS], mybir.dt.float32, kind="ExternalOutput")
    t16_d = nc.dram_tensor("t16", [TSIZE, 1], mybir.dt.float32, kind="Internal")

    f32 = mybir.dt.float32
    AL = mybir.AluOpType

    def sap(ap, pattern, off=0):
        v = ap.copy()
        v.ap = type(v.ap)(pattern)
        v.offset = v.offset + off
        return v

    with tile.TileContext(nc) as tc:
        with ExitStack() as ctx:
            bpool = ctx.enter_context(tc.tile_pool(name="bpool", bufs=2))
            pool = ctx.enter_context(tc.tile_pool(name="pool", bufs=2))
            xpool = ctx.enter_context(tc.tile_pool(name="xpool", bufs=2))

            # ---- phase 1: build T16 ----
            for blk in range(32):
                x0b = blk * 4
                gtiles = []
                for j in range(4):
                    # gj[p] = gt row (p+j), 7 x-planes x0b..x0b+6
                    gj = bpool.tile([P, 7, GRID], f32, tag=f"g{j}")
                    src = sap(
                        gt_d[:],
                        [[G2, P], [GRID, 7], [1, GRID]],
                        j * G2 + x0b * GRID,
                    )
                    nc.sync.dma_start(gj[:], src)
                    gtiles.append(gj)
                S = bpool.tile([P, 4, NZ, CELL], f32, tag="S")
                for i in range(4):
                    for j in range(4):
                        nc.vector.tensor_copy(
                            S[:, :, :, i * 4 + j],
                            gtiles[j][:, i : i + 4, 0:NZ],
                        )
                for b in range(4):
                    dst = sap(t16_d[:], [[ROW, P], [1, ROW]], (x0b + b) * PLANE)
                    nc.sync.dma_start(dst, S[:, b])

            # Indirect gathers must not race the table build (the DRAM
            # dependency through t16_d is not reliably tracked).
            tc.strict_bb_all_engine_barrier()

            # ---- phase 2: gather + contract ----
            for cc in range(nchunks):
                c = cc % NCHUNK
                pts_t = pool.tile([P, NC, 3], f32, tag="pts")
                src = sap(pts_d[:], [[SLOTS * 3, P], [3, NC], [1, 3]], c * NC * 3)
                nc.sync.dma_start(pts_t[:], src)

                t_t = pool.tile([P, NC, 3], f32, tag="t")
                nc.vector.tensor_scalar_add(t_t[:], pts_t[:], 1.0)
                # floor via round-to-nearest(t + 2^23) - 2^23, fix up rounds
                r_t = pool.tile([P, NC, 3], f32, tag="r")
                nc.vector.tensor_scalar(
                    r_t[:], t_t[:], 8388608.0, 8388608.0, op0=AL.add, op1=AL.subtract
                )
                gt_t = pool.tile([P, NC, 3], f32, tag="gt")
                nc.vector.tensor_tensor(gt_t[:], r_t[:], t_t[:], op=AL.is_gt)
                tif_t = pool.tile([P, NC, 3], f32, tag="tif")
                nc.vector.tensor_sub(tif_t[:], r_t[:], gt_t[:])
                frac_t = pool.tile([P, NC, 3], f32, tag="frac")
                nc.vector.tensor_sub(frac_t[:], t_t[:], tif_t[:])

                # spline weights W[p, n, dim, tap]
                W = pool.tile([P, NC, 3, 4], f32, tag="W")
                omx = pool.tile([P, NC, 3], f32, tag="omx")
                nc.vector.tensor_scalar(
                    omx[:], frac_t[:], -1.0, -1.0, op0=AL.mult, op1=AL.subtract
                )  # 1 - x
                x2 = pool.tile([P, NC, 3], f32, tag="x2")
                nc.vector.tensor_mul(x2[:], frac_t[:], frac_t[:])
                x3 = pool.tile([P, NC, 3], f32, tag="x3")
                nc.vector.tensor_mul(x3[:], x2[:], frac_t[:])
                o2 = pool.tile([P, NC, 3], f32, tag="o2")
                nc.vector.tensor_mul(o2[:], omx[:], omx[:])
                o3 = pool.tile([P, NC, 3], f32, tag="o3")
                nc.vector.tensor_mul(o3[:], o2[:], omx[:])
                SIX = 1.0 / 6.0
                nc.vector.tensor_scalar_mul(W[:, :, :, 0], o3[:], SIX)
                nc.vector.tensor_scalar_mul(W[:, :, :, 3], x3[:], SIX)
                c1a = pool.tile([P, NC, 3], f32, tag="c1a")
                nc.vector.scalar_tensor_tensor(
                    c1a[:], x3[:], 0.5, x2[:], op0=AL.mult, op1=AL.subtract
                )
                nc.vector.tensor_scalar_add(W[:, :, :, 1], c1a[:], 2.0 / 3.0)
                c2a = pool.tile([P, NC, 3], f32, tag="c2a")
                nc.vector.scalar_tensor_tensor(
                    c2a[:], o3[:], 0.5, o2[:], op0=AL.mult, op1=AL.subtract
                )
                nc.vector.tensor_scalar_add(W[:, :, :, 2], c2a[:], 2.0 / 3.0)

                # flat idx = ((bx-1)*128 + (by-1))*2096 + (bz-1)*16
                #          = (bx*16768 + by*131 + bz - 16900) * 16; exact in f32
                bx = tif_t[:, :, 0]
                by = tif_t[:, :, 1]
                bz = tif_t[:, :, 2]
                f1 = pool.tile([P, NC], f32, tag="f1")
                nc.vector.scalar_tensor_tensor(
                    f1[:], by, float(NZ), bz, op0=AL.mult, op1=AL.add
                )
                f2 = pool.tile([P, NC], f32, tag="f2")
                nc.vector.scalar_tensor_tensor(
                    f2[:], bx, float(NX * NZ), f1[:], op0=AL.mult, op1=AL.add
                )
                basef = pool.tile([P, NC], f32, tag="basef")
                nc.vector.tensor_scalar(
                    basef[:], f2[:], float(-(NX * NZ + NZ + 1)), float(CELL),
                    op0=AL.add, op1=AL.mult,
                )
                idxi = pool.tile([P, NC], mybir.dt.int32, tag="idxi")
                nc.vector.tensor_copy(idxi[:], basef[:])

                # gather: one 64-f32 (256B) descriptor per point, layout (k,i,j)
                # NB: X must stay 3D - the indirect out AP has to be
                # [partition, contiguous-run]; >2D out APs gather garbage.
                X = xpool.tile([P, NC, 64], f32, tag="X")
                for n in range(NC):
                    nc.gpsimd.indirect_dma_start(
                        out=X[:, n],
                        out_offset=None,
                        in_=t16_d[:],
                        in_offset=bass.IndirectOffsetOnAxis(
                            ap=idxi[:, n : n + 1], axis=0
                        ),
                    )

                # X[(k,i),j] * wy[j] -> reduce j -> A[(k,i)]
                m1 = xpool.tile([P, NC, 16, 4], f32, tag="m1")
                Xv = sap(X[:], [[NC * 64, P], [64, NC], [4, 16], [1, 4]])
                wy = sap(W[:], [[NC * 12, P], [12, NC], [0, 16], [1, 4]], 1 * 4)
                nc.vector.tensor_tensor(m1[:], Xv, wy, op=AL.mult)
                A = pool.tile([P, NC, 16], f32, tag="A")
                nc.vector.tensor_reduce(
                    A[:].rearrange("p n g -> p (n g)"),
                    m1[:].rearrange("p n g j -> p (n g) j"),
                    axis=mybir.AxisListType.X,
                    op=AL.add,
                )
                # A[(k,i)] * wx[i] -> reduce i -> B[k]
                m2 = pool.tile([P, NC, 4, 4], f32, tag="m2")
                Av = sap(A[:], [[NC * 16, P], [16, NC], [4, 4], [1, 4]])
                wx = sap(W[:], [[NC * 12, P], [12, NC], [0, 4], [1, 4]], 0)
                nc.vector.tensor_tensor(m2[:], Av, wx, op=AL.mult)
                B = pool.tile([P, NC, 4], f32, tag="B")
                nc.vector.tensor_reduce(
                    B[:].rearrange("p n k -> p (n k)"),
                    m2[:].rearrange("p n k i -> p (n k) i"),
                    axis=mybir.AxisListType.X,
                    op=AL.add,
                )
                # B[k] * wz[k] -> reduce k -> v
                m3 = pool.tile([P, NC, 4], f32, tag="m3")
                wz = sap(W[:], [[NC * 12, P], [12, NC], [1, 4]], 2 * 4)
                nc.vector.tensor_tensor(m3[:], B[:], wz, op=AL.mult)
                v = pool.tile([P, NC], f32, tag="v")
                nc.vector.tensor_reduce(
                    v[:], m3[:], axis=mybir.AxisListType.X, op=AL.add
                )

                dst = sap(out_d[:], [[SLOTS, P], [1, NC]], c * NC)
                nc.sync.dma_start(dst, v[:])

    nc.compile()
    return nc


def host_inputs(pts, control_pts):
    """Per-core in_maps from full inputs."""
    pts = np.ascontiguousarray(pts, dtype=np.float32)
    g3 = np.ascontiguousarray(control_pts, np.float32).reshape(GRID, GRID, GRID)
    gt = np.ascontiguousarray(g3.transpose(1, 0, 2)).reshape(GRID * G2, 1)
    in_maps = []
    for k in range(8):
        sl = pts[k * NPTS_CORE : (k + 1) * NPTS_CORE]
        pad = np.zeros((NPAD_CORE, 3), np.float32)
        pad[: sl.shape[0]] = sl
        in_maps.append({"pts": pad, "gt": gt})
    return in_maps


def kernel(pts: np.ndarray, control_pts: np.ndarray) -> np.ndarray:
    from concourse.bass_utils import run_bass_kernel_spmd

    if "nc" not in _CACHE:
        _CACHE["nc"] = _build_program()
    nc = _CACHE["nc"]
    in_maps = host_inputs(pts, control_pts)
    res = run_bass_kernel_spmd(nc, in_maps, core_ids=list(range(8)))
    outs = []
    for k in range(8):
        o = res.results[k]["out"].reshape(NPAD_CORE)
        outs.append(o[:NPTS_CORE])
    return np.concatenate(outs).reshape(-1, 1)
